# revision 14
# baseline (speedup 1.0000x reference)
"""Multi-head attention (B=4, N=2048, C=1024, H=16) on 8 TRN2 NeuronCores.

Sharding: tensor-parallel over heads; each core owns H/8 = 2 heads.
  - qkv: w_qkv column-sharded by head group, x replicated (pre-transposed
    as xT). Q^T/K^T/V^T strips from 512-wide K=128 chains; V re-transposed
    on the PE with a ones column appended (softmax sums ride PV for free).
  - attention: per 512-wide q-chunk (4 per batch), 16 k-tile iterations.
    Per iteration the two heads' K=64 score matmuls are emitted adjacent
    into the two banks of one PSUM tile so they co-execute in disjoint PE
    row halves (measured 2x); one 1024-wide exp instruction covers both
    heads; two K=128 PV accumulations follow.
  - softmax normalize: po is copied out of PSUM immediately (frees the
    banks for the next chunk), 1/sums via the DVE reciprocal_approx (no
    ScalarE work - ACT does exps only), broadcast via a DRAM bounce.
  - proj: one 128KB AllToAll per q-chunk redistributes attention output;
    chunk-pair proj groups (M=128) run as filler; the last batch's chunks
    are proj'd inside its own loop so the tail is one A2A + M=64 proj.

Scheduling: single software-pipelined stream; qkv of batch b+1, V
transposes, and proj groups interleave the attention loop as filler.
"""


import numpy as np

import concourse.bass as bass
import concourse.mybir as mybir
import concourse.tile as tile
from concourse import bacc
from concourse.bass_utils import run_bass_kernel_spmd
from concourse.masks import make_identity

F32 = mybir.dt.float32
BF16 = mybir.dt.bfloat16

B, N, C, H = 4, 2048, 1024, 16
NCORES = 8


def build_nc(b_sz=B, n_sz=N, c_sz=C, h_sz=H, ncores=NCORES):
    HD = c_sz // h_sz                # 64
    HPC = h_sz // ncores             # 2 heads per core
    WC = HPC * HD                    # 128
    RB = n_sz // ncores              # 256 out rows per (core, batch)
    SCALE = float(HD) ** -0.5

    RCW = 512                        # qkv row-chunk width
    NRC = n_sz // RCW
    QCW = 512                        # attention q-chunk width
    NQC = n_sz // QCW                # 4
    NKT = n_sz // 128                # 16
    CCH = c_sz // 128                # 8
    NCOL = c_sz // 512               # 2
    CRB = RB // NQC                  # 64 rows per (core, chunk)
    NIT = NQC * NKT                  # 64

    assert n_sz % QCW == 0 and n_sz % RCW == 0 and RB % CRB == 0
    assert WC == 128 and HD == 64 and CRB == 64

    # Pin every activation to the one table set containing both Exp and
    # Ln, so the Ln/Exp softmax-reciprocal never thrashes ACT table loads.
    from concourse import hw_specs
    tables = hw_specs.get_activation_tables("gen3")
    for name, fns in tables.items():
        if name != "natural_log_exp_and_others":
            fns.discard(mybir.ActivationFunctionType.Exp)
            fns.discard(mybir.ActivationFunctionType.Ln)

    nc = bacc.Bacc(
        "TRN2", target_bir_lowering=False, debug=False, num_devices=ncores
    )

    xT = nc.dram_tensor(
        "xT", [b_sz * NRC, 128, CCH, RCW], BF16, kind="ExternalInput").ap()
    wqkv = nc.dram_tensor(
        "wqkv", [128, CCH, 3 * WC], BF16, kind="ExternalInput").ap()
    wproj = nc.dram_tensor(
        "wproj", [128, CCH, c_sz], BF16, kind="ExternalInput").ap()
    bproj = nc.dram_tensor("bproj", [c_sz], F32, kind="ExternalInput").ap()
    out = nc.dram_tensor("out", [b_sz, RB, c_sz], F32, kind="ExternalOutput").ap()

    with tile.TileContext(nc) as tc:
        from contextlib import ExitStack

        with ExitStack() as ctx:
            singles = ctx.enter_context(tc.tile_pool(name="singles", bufs=1))
            xpool = ctx.enter_context(tc.tile_pool(name="xpool", bufs=4))
            qt_pool = ctx.enter_context(tc.tile_pool(name="qt", bufs=2))
            kt_pool = ctx.enter_context(tc.tile_pool(name="kt", bufs=2))
            vt_pool = ctx.enter_context(tc.tile_pool(name="vt", bufs=2))
            vaug_pool = ctx.enter_context(tc.tile_pool(name="vaug", bufs=2))
            es_pool = ctx.enter_context(tc.tile_pool(name="es", bufs=4))
            ot_pool = ctx.enter_context(tc.tile_pool(name="ot", bufs=2))
            yep_pool = ctx.enter_context(tc.tile_pool(name="yep", bufs=4))
            rcb_pool = ctx.enter_context(tc.tile_pool(name="rcb", bufs=2))
            bc_pool = ctx.enter_context(tc.tile_pool(name="bc", bufs=4))
            recv_pool = ctx.enter_context(tc.tile_pool(name="recv", bufs=2))
            y_pool = ctx.enter_context(tc.tile_pool(name="y", bufs=3))
            sc_pool = ctx.enter_context(
                tc.tile_pool(name="scp", bufs=2, space="PSUM")
            )
            po_pool = ctx.enter_context(
                tc.tile_pool(name="pop", bufs=2, space="PSUM")
            )
            misc_pool = ctx.enter_context(
                tc.tile_pool(name="mip", bufs=2, space="PSUM")
            )
            dram_in = ctx.enter_context(
                tc.tile_pool(name="a2a_in", bufs=3, space="DRAM")
            )
            dram_out = ctx.enter_context(
                tc.tile_pool(name="a2a_out", bufs=6, space="DRAM")
            )
            dram_bc = ctx.enter_context(
                tc.tile_pool(name="bc_dram", bufs=3, space="DRAM")
            )

            # ---- constants / weights ----
            w_sb = singles.tile([128, CCH, 3 * WC], BF16)
            nc.sync.dma_start(out=w_sb, in_=wqkv)
            identity = singles.tile([128, 128], BF16)
            make_identity(nc, identity)

            proj_consts = {}

            def ensure_proj_consts():
                if proj_consts:
                    return
                wp_sb = singles.tile([128, CCH, c_sz], BF16)
                nc.sync.dma_start(out=wp_sb, in_=wproj)
                b_sb = singles.tile([128, c_sz], F32)
                nc.gpsimd.dma_start(
                    out=b_sb,
                    in_=bass.AP(
                        tensor=bproj.tensor,
                        offset=bproj.offset,
                        ap=[[0, 128]] + list(bproj.ap),
                    ),
                )
                proj_consts["wp"] = wp_sb
                proj_consts["b"] = b_sb

            def chunk_order(b):
                # last batch: process chunks 2..N,0,1 so the tail exposes
                # only chunk 1's AllToAll
                if b == b_sz - 1 and NQC >= 4:
                    return list(range(2, NQC)) + [0, 1]
                return list(range(NQC))

            # per-batch state
            strips = {}       # b -> (QT, KT, VT)
            x_tiles = {}      # (b, rc) -> x_sb
            vaugs = {}        # b -> Vaug
            a2a_chunks = {}   # (b, qc) -> a_out
            oths = {}         # b -> [oTh per head]

            last_b = b_sz - 1

            # ---------- emission helpers ----------

            def emit_strip_group(b, rc, si):
                """One qkv strip-group: (maybe x DMA +) 8 matmuls + copy."""
                if b not in strips:
                    QT = qt_pool.tile([WC, n_sz], BF16, name="QT")
                    KTs = kt_pool.tile([WC, n_sz], BF16, name="KT")
                    VT = vt_pool.tile([WC, n_sz], BF16, name="VT")
                    strips[b] = (QT, KTs, VT)
                QT, KTs, VT = strips[b]
                if (b, rc) not in x_tiles:
                    x_sb = xpool.tile([128, CCH, RCW], BF16)
                    nc.sync.dma_start(out=x_sb, in_=xT[b * NRC + rc])
                    x_tiles[(b, rc)] = x_sb
                x_sb = x_tiles[(b, rc)]
                strip, base = ((VT, 2 * WC), (QT, 0), (KTs, WC))[si]
                ps = misc_pool.tile([128, RCW], F32, tag="mi")
                for cc in range(CCH):
                    nc.tensor.matmul(
                        ps,
                        lhsT=w_sb[:, cc, base:base + WC],
                        rhs=x_sb[:, cc, :],
                        start=(cc == 0),
                        stop=(cc == CCH - 1),
                    )
                nc.vector.tensor_copy(strip[:, rc * RCW:(rc + 1) * RCW], ps)

            def emit_transpose(b, kt):
                """V^T -> V natural for one 128-col k-tile (both heads)."""
                if b not in vaugs:
                    Vaug = vaug_pool.tile([128, HPC * NKT, HD + 1], BF16)
                    nc.vector.memset(Vaug[:, :, HD], 1.0)
                    vaugs[b] = Vaug
                Vaug = vaugs[b]
                VT = strips[b][2]
                pst = misc_pool.tile([128, 128], BF16, tag="mi")
                nc.tensor.transpose(
                    pst, VT[:, kt * 128:(kt + 1) * 128], identity
                )
                nc.vector.tensor_copy(
                    Vaug[:, kt::NKT, 0:HD], pst.rearrange(
                        "p (h d) -> p h d", h=HPC
                    )
                )

            def emit_proj_pair(b, pair, stage):
                """Chunk-pair proj, M=128. stage 0: recv DMA; 1,2: ncol
                matmul group + bias + out DMA."""
                ensure_proj_consts()
                if stage == 0:
                    recv = recv_pool.tile([WC, ncores, 2 * CRB], BF16,
                                          tag="recv")
                    for i in range(2):
                        src = a2a_chunks[(b, 2 * pair + i)]
                        nc.sync.dma_start(
                            out=recv[:, :, i * CRB:(i + 1) * CRB],
                            in_=src.rearrange("j p q -> p j q"),
                        )
                    return recv
                recv, ncol = stage
                wp_sb = proj_consts["wp"]
                b_sb = proj_consts["b"]
                yp = misc_pool.tile([128, 512], F32, tag="mi")
                for j in range(ncores):
                    nc.tensor.matmul(
                        yp,
                        lhsT=recv[:, j, :],
                        rhs=wp_sb[:, j, ncol * 512:(ncol + 1) * 512],
                        start=(j == 0),
                        stop=(j == ncores - 1),
                    )
                y_sb = y_pool.tile([128, 512], F32)
                nc.vector.tensor_add(
                    y_sb, yp, b_sb[:, ncol * 512:(ncol + 1) * 512]
                )
                nc.sync.dma_start(
                    out=out[
                        b,
                        pair * 2 * CRB: (pair + 1) * 2 * CRB,
                        ncol * 512:(ncol + 1) * 512,
                    ],
                    in_=y_sb,
                )

            def emit_proj_single(b, qc, stage):
                """Single-chunk proj, M=64 (used near the tail)."""
                ensure_proj_consts()
                if stage == 0:
                    recv = recv_pool.tile([WC, ncores, CRB], BF16, tag="recv1")
                    src = a2a_chunks[(b, qc)]
                    nc.sync.dma_start(
                        out=recv, in_=src.rearrange("j p q -> p j q")
                    )
                    return recv
                recv, ncol = stage
                wp_sb = proj_consts["wp"]
                b_sb = proj_consts["b"]
                yp = misc_pool.tile([CRB, 512], F32, tag="mi")
                for j in range(ncores):
                    nc.tensor.matmul(
                        yp,
                        lhsT=recv[:, j, :],
                        rhs=wp_sb[:, j, ncol * 512:(ncol + 1) * 512],
                        start=(j == 0),
                        stop=(j == ncores - 1),
                    )
                y_sb = y_pool.tile([CRB, 512], F32)
                nc.vector.tensor_add(
                    y_sb, yp, b_sb[0:CRB, ncol * 512:(ncol + 1) * 512]
                )
                nc.sync.dma_start(
                    out=out[
                        b,
                        qc * CRB:(qc + 1) * CRB,
                        ncol * 512:(ncol + 1) * 512,
                    ],
                    in_=y_sb,
                )

            def emit_boundary(b, qc, po):
                """Chunk epilogue: free PSUM, 1/sums, normalize, ship A2A."""
                # copy po out of PSUM so the next chunk's PV can start
                yc = [yep_pool.tile([HD + 1, QCW], F32, tag="yep",
                                    name=f"yc{h}") for h in range(HPC)]
                for h in range(HPC):
                    nc.vector.tensor_copy(yc[h], po[h])
                # reciprocal of the sums rows (DVE, in place) + pack to a
                # partition-64-aligned tile for the DRAM bounce
                # 1/sums = exp(-ln(sums)) on ACT (both functions pinned to
                # one table set); Exp emits bf16 for the broadcast bounce
                rcb = rcb_pool.tile([HD + 1, HPC, QCW], F32, tag="rcb")
                recb = rcb_pool.tile([HD + 1, HPC, QCW], BF16, tag="recb")
                for h in range(HPC):
                    nc.scalar.activation(
                        rcb[HD:HD + 1, h, :], yc[h][HD:HD + 1, :],
                        mybir.ActivationFunctionType.Ln,
                    )
                    nc.scalar.activation(
                        recb[HD:HD + 1, h, :], rcb[HD:HD + 1, h, :],
                        mybir.ActivationFunctionType.Exp,
                        scale=-1.0,
                    )
                rvd = dram_bc.tile([1, HPC, QCW], BF16, tag="rvd")
                nc.gpsimd.dma_start(out=rvd, in_=recb[HD:HD + 1])
                oth = oths[b]
                for h in range(HPC):
                    bc = bc_pool.tile([HD, QCW], BF16)
                    rvh = rvd[0, h]
                    nc.gpsimd.dma_start(
                        out=bc,
                        in_=bass.AP(
                            tensor=rvh.tensor,
                            offset=rvh.offset,
                            ap=[[0, HD]] + list(rvh.ap),
                        ),
                    )
                    nc.vector.tensor_mul(
                        oth[h][:, qc * QCW:(qc + 1) * QCW], yc[h][0:HD], bc
                    )
                # ship
                a_in = dram_in.tile([ncores, WC, CRB], BF16, tag="ain",
                                    name=f"ain{qc}")
                for h in range(HPC):
                    src = oth[h][:, qc * QCW:(qc + 1) * QCW]
                    nc.sync.dma_start(
                        out=a_in[:, h * HD:(h + 1) * HD, :].rearrange(
                            "j d q -> d j q"
                        ),
                        in_=src.rearrange("d (j q) -> d j q", j=ncores),
                    )
                a_out = dram_out.tile([ncores, WC, CRB], BF16, tag="aout",
                                      name=f"aout{qc}")
                a2a_chunks[(b, qc)] = a_out
                nc.gpsimd.collective_compute(
                    "AllToAll",
                    mybir.AluOpType.bypass,
                    replica_groups=[list(range(ncores))],
                    ins=[a_in.opt()],
                    outs=[a_out.opt()],
                )

            # ---------- filler schedules ----------

            def filler_schedule(b):
                """iteration k -> list of closures."""
                sched = {k: [] for k in range(NIT + 1)}
                if b + 1 < b_sz:
                    ngroups = 3 * NRC
                    lo = 12 if b == 0 else 2
                    hi = max(NIT - 12, lo + ngroups)
                    for g in range(ngroups):
                        rc, si = divmod(g, 3)
                        k = lo + (g * (hi - lo)) // ngroups
                        sched[min(k, NIT)].append(
                            lambda rc=rc, si=si: emit_strip_group(b + 1, rc, si)
                        )
                    q4 = max(NIT // 4, 1)
                    base = max(hi, NIT - q4)  # never before the V strips
                    for kt in range(NKT):
                        k = base + (kt * max(NIT - base, 1)) // NKT
                        sched[min(k, NIT)].append(
                            lambda kt=kt: emit_transpose(b + 1, kt)
                        )

                def add_pair(bp, pair, k0):
                    state = {}

                    def recv(state=state, bp=bp, pair=pair):
                        state["r"] = emit_proj_pair(bp, pair, 0)

                    sched[min(k0, NIT)].append(recv)
                    for ncol in range(NCOL):
                        sched[min(k0 + 2 + 2 * ncol, NIT)].append(
                            lambda ncol=ncol, state=state, bp=bp, pair=pair:
                            emit_proj_pair(bp, pair, (state["r"], ncol))
                        )

                def add_single(bp, qc, k0):
                    state = {}

                    def recv(state=state, bp=bp, qc=qc):
                        state["r"] = emit_proj_single(bp, qc, 0)

                    sched[min(k0, NIT)].append(recv)
                    for ncol in range(NCOL):
                        sched[min(k0 + 2 + 2 * ncol, NIT)].append(
                            lambda ncol=ncol, state=state, bp=bp, qc=qc:
                            emit_proj_single(bp, qc, (state["r"], ncol))
                        )

                npair = NQC // 2
                if b - 1 >= 0:
                    # all of the previous batch's proj pairs, late enough
                    # that the real A2A latency can't head-block the PE
                    # queue (the compile-time scheduler trusts its own
                    # optimistic collective model)
                    for p in range(npair):
                        add_pair(b - 1, npair - 1 - p, 36 + 8 * p)
                if b == last_b:
                    # own chunks: pairs processed (and shipped) early in
                    # this batch, then a single for the second-to-last
                    # processed chunk; the last one goes in the tail
                    order = chunk_order(b)
                    for p in range(npair - 1):
                        qa, qb_ = order[2 * p], order[2 * p + 1]
                        assert qb_ == qa + 1 and qa % 2 == 0
                        # runs inside the tail's A2A wait (free PE time)
                        add_pair(b, qa // 2, NIT - 2 * (npair - 1 - p))
                    if NQC >= 2:
                        add_single(b, order[NQC - 2], NIT - 2)
                return sched

            # ---------- main stream ----------

            # prefetch batch 0's x chunks on parallel DMA queues
            dmaq = [nc.gpsimd, nc.scalar, nc.sync, nc.scalar]
            for rc in range(NRC):
                x_sb = xpool.tile([128, CCH, RCW], BF16)
                dmaq[rc % len(dmaq)].dma_start(out=x_sb, in_=xT[rc])
                x_tiles[(0, rc)] = x_sb
            KPG = NKT // NRC
            for si in (2, 1, 0):  # K,Q first - the first scores need them
                emit_strip_group(0, 0, si)
            for kt in range(KPG):
                emit_transpose(0, kt)
            ensure_proj_consts()

            for b in range(b_sz):
                QT, KTs, VT = strips[b]
                Vaug = vaugs[b]
                oths[b] = [ot_pool.tile([HD, n_sz], BF16, name=f"oTh{h}")
                           for h in range(HPC)]
                sched = filler_schedule(b)
                if b == 0:
                    # rest of batch 0's own qkv, paced just ahead of use
                    for r in range(1, NRC):
                        for si in range(3):
                            sched[3 * (r - 1) + si].insert(
                                si, (lambda r=r, si=si:
                                     emit_strip_group(0, r, si)))
                        for kt in range(KPG * r, KPG * (r + 1)):
                            sched[3 * r].insert(
                                3, lambda kt=kt: emit_transpose(0, kt))
                es_hist = {}
                po_cur = None

                qc_ord = chunk_order(b)

                def emit_pv(k):
                    nonlocal po_cur
                    qp, ktp = divmod(k, NKT)
                    qp = qc_ord[qp]
                    if ktp == 0:
                        po_cur = po_new[0]
                    es_prev = es_hist.pop(k)
                    for h in range(HPC):
                        nc.tensor.matmul(
                            po_cur[h],
                            lhsT=Vaug[:, h * NKT + ktp, :],
                            rhs=es_prev[:, h, :],
                            start=(ktp == 0), stop=(ktp == NKT - 1),
                        )
                    if ktp == NKT - 1:
                        emit_boundary(b, qp, po_cur)

                def emit_scores(k):
                    qc, kt = divmod(k, NKT)
                    qc = qc_ord[qc]
                    if kt == 0:
                        po_new[0] = [po_pool.tile([HD + 1, QCW], F32,
                                                  tag="po", name=f"po{h}")
                                     for h in range(HPC)]
                    sc = sc_pool.tile([128, HPC, QCW], F32, tag="sc")
                    for h in range(HPC):
                        nc.tensor.matmul(
                            sc[:, h, :],
                            lhsT=KTs[h * HD:(h + 1) * HD,
                                     kt * 128:(kt + 1) * 128],
                            rhs=QT[h * HD:(h + 1) * HD,
                                   qc * QCW:(qc + 1) * QCW],
                            start=True, stop=True,
                        )
                    sc_hist[k] = sc

                def emit_exp(k):
                    es = es_pool.tile([128, HPC, QCW], BF16, tag="es")
                    nc.scalar.activation(
                        es, sc_hist.pop(k), mybir.ActivationFunctionType.Exp,
                        scale=SCALE,
                    )
                    es_hist[k] = es

                # iterations processed in groups of 2 so the PE runs 4
                # same-geometry score matmuls, then 4 PVs (geometry
                # switches flush the weight-load pipeline)
                po_new = [None]
                sc_hist = {}
                for g in range(NIT // 2 + 1):
                    kE = 2 * g
                    boundary = kE >= 1 and kE % NKT == 0
                    if boundary:
                        # the chunk's last PVs + epilogue/ship go FIRST so
                        # the A2A chain starts before this group's scores
                        emit_pv(kE - 2)
                        emit_pv(kE - 1)
                    for k in (kE, kE + 1):
                        if k < NIT:
                            emit_scores(k)
                    for k in (kE, kE + 1):
                        if k < NIT:
                            emit_exp(k)
                    if not boundary:
                        for kp in (kE - 2, kE - 1):
                            if kp >= 0:
                                emit_pv(kp)
                    for k in (kE, kE + 1):
                        if k <= NIT:
                            for fn in sched[k]:
                                fn()

                for rc in range(NRC):
                    x_tiles.pop((b, rc), None)

            # ---------- tail: final chunk's proj ----------
            last_qc = chunk_order(last_b)[NQC - 1]
            recv = emit_proj_single(last_b, last_qc, 0)
            for ncol in range(NCOL):
                emit_proj_single(last_b, last_qc, (recv, ncol))

    nc.compile()
    return nc


def shard_inputs(x, w_qkv, w_proj, b_proj, b_sz=B, n_sz=N, c_sz=C, h_sz=H,
                 ncores=NCORES):
    """Build per-core input maps from the full inputs."""
    import ml_dtypes

    mm_np = ml_dtypes.bfloat16
    HPC = h_sz // ncores
    HD = c_sz // h_sz
    x = np.asarray(x, dtype=np.float32)
    w_qkv = np.asarray(w_qkv, dtype=np.float32).astype(mm_np)
    w_proj = np.ascontiguousarray(np.asarray(w_proj, dtype=np.float32)
                                  .astype(mm_np))
    b_proj = np.ascontiguousarray(np.asarray(b_proj, dtype=np.float32))

    RCW = 512
    NRC = n_sz // RCW
    CCH = c_sz // 128
    # x chunk (b, rc) pre-transposed to [128, CCH, RCW], chunk-contiguous
    xT = x.reshape(b_sz * n_sz, c_sz).T.astype(mm_np)          # [C, B*N]
    xTc = np.ascontiguousarray(
        xT.reshape(CCH, 128, b_sz * NRC, RCW).transpose(2, 1, 0, 3)
    )
    wp_c = np.ascontiguousarray(
        w_proj.reshape(CCH, 128, c_sz).transpose(1, 0, 2)
    )
    w4 = w_qkv.reshape(c_sz, 3, h_sz, HD)
    in_maps = []
    for c in range(ncores):
        wc = w4[:, :, c * HPC:(c + 1) * HPC, :].reshape(c_sz, 3 * HPC * HD)
        wc_c = np.ascontiguousarray(
            wc.reshape(CCH, 128, 3 * HPC * HD).transpose(1, 0, 2)
        )
        in_maps.append(
            {"xT": xTc, "wqkv": wc_c, "wproj": wp_c, "bproj": b_proj}
        )
    return in_maps


def assemble_output(results, b_sz=B, n_sz=N, c_sz=C, ncores=NCORES):
    RB = n_sz // ncores
    QCW = 512
    NQC = n_sz // QCW
    CRB = RB // NQC
    full = np.empty((b_sz, n_sz, c_sz), dtype=np.float32)
    for r in range(ncores):
        o = results[r]["out"]
        for b in range(b_sz):
            for qc in range(NQC):
                full[b, qc * QCW + r * CRB: qc * QCW + (r + 1) * CRB, :] \
                    = o[b, qc * CRB:(qc + 1) * CRB, :]
    return full


def run(x, w_qkv, w_proj, b_proj, trace=False, **run_kwargs):
    nc = build_nc()
    in_maps = shard_inputs(x, w_qkv, w_proj, b_proj)
    last_err = None
    for attempt in range(3):
        try:
            res = run_bass_kernel_spmd(
                nc, in_maps, core_ids=list(range(NCORES)), trace=trace,
                **run_kwargs
            )
            return assemble_output(res.results), res
        except Exception as e:  # transient device wedges happen; retry
            last_err = e
            import time
            time.sleep(10)
    raise last_err


def kernel(x, w_qkv, w_proj, b_proj):
    out, _ = run(x, w_qkv, w_proj, b_proj)
    return out


# revision 15
# speedup vs baseline: 1.0299x; 1.0299x over previous
"""Multi-head attention (B=4, N=2048, C=1024, H=16) on 8 TRN2 NeuronCores.

Sharding: tensor-parallel over heads; each core owns H/8 = 2 heads.
  - qkv: w_qkv column-sharded by head group, x replicated (pre-transposed
    as xT). Q^T/K^T/V^T strips from 512-wide K=128 chains; V re-transposed
    on the PE with a ones column appended (softmax sums ride PV for free).
  - attention: per 512-wide q-chunk (4 per batch), 16 k-tile iterations.
    Per iteration the two heads' K=64 score matmuls are emitted adjacent
    into the two banks of one PSUM tile so they co-execute in disjoint PE
    row halves (measured 2x); one 1024-wide exp instruction covers both
    heads; two K=128 PV accumulations follow.
  - softmax normalize: po is copied out of PSUM immediately (frees the
    banks for the next chunk), 1/sums = exp(-ln(sums)) on ScalarE (both
    functions pinned to one activation table), broadcast via a DRAM
    bounce with a 0-stride partition read.
  - proj: one 128KB AllToAll per q-chunk redistributes attention output;
    chunk-pair proj groups (M=128) run as filler in the NEXT batch's
    loop, far enough in that the compile-time scheduler's optimistic
    collective model cannot head-block the in-order PE queue on a real
    ~15us A2A; the last batch processes its chunks in order 2,3,0,1 and
    fills its tail A2A wait with the deferred pair proj.

Scheduling: single software-pipelined stream; qkv of batch b+1, V
transposes, and proj groups interleave the attention loop as filler.
"""


import numpy as np

import concourse.bass as bass
import concourse.mybir as mybir
import concourse.tile as tile
from concourse import bacc
from concourse.bass_utils import run_bass_kernel_spmd
from concourse.masks import make_identity

F32 = mybir.dt.float32
BF16 = mybir.dt.bfloat16

B, N, C, H = 4, 2048, 1024, 16
NCORES = 8


def build_nc(b_sz=B, n_sz=N, c_sz=C, h_sz=H, ncores=NCORES):
    HD = c_sz // h_sz                # 64
    HPC = h_sz // ncores             # 2 heads per core
    WC = HPC * HD                    # 128
    RB = n_sz // ncores              # 256 out rows per (core, batch)
    SCALE = float(HD) ** -0.5

    RCW = 512                        # qkv row-chunk width
    NRC = n_sz // RCW
    QCW = 512                        # attention q-chunk width
    NQC = n_sz // QCW                # 4
    NKT = n_sz // 128                # 16
    CCH = c_sz // 128                # 8
    NCOL = c_sz // 512               # 2
    CRB = RB // NQC                  # 64 rows per (core, chunk)
    NIT = NQC * NKT                  # 64

    assert n_sz % QCW == 0 and n_sz % RCW == 0 and RB % CRB == 0
    assert WC == 128 and HD == 64 and CRB == 64

    # Pin every activation to the one table set containing both Exp and
    # Ln, so the Ln/Exp softmax-reciprocal never thrashes ACT table loads.
    from concourse import hw_specs
    tables = hw_specs.get_activation_tables("gen3")
    for name, fns in tables.items():
        if name != "natural_log_exp_and_others":
            fns.discard(mybir.ActivationFunctionType.Exp)
            fns.discard(mybir.ActivationFunctionType.Ln)

    nc = bacc.Bacc(
        "TRN2", target_bir_lowering=False, debug=False, num_devices=ncores
    )

    xT = nc.dram_tensor(
        "xT", [b_sz * NRC, 128, CCH, RCW], BF16, kind="ExternalInput").ap()
    wqkv = nc.dram_tensor(
        "wqkv", [128, CCH, 3 * WC], BF16, kind="ExternalInput").ap()
    wproj = nc.dram_tensor(
        "wproj", [128, CCH, c_sz], BF16, kind="ExternalInput").ap()
    bproj = nc.dram_tensor("bproj", [c_sz], F32, kind="ExternalInput").ap()
    out = nc.dram_tensor("out", [b_sz, RB, c_sz], F32, kind="ExternalOutput").ap()

    with tile.TileContext(nc) as tc:
        from contextlib import ExitStack

        with ExitStack() as ctx:
            singles = ctx.enter_context(tc.tile_pool(name="singles", bufs=1))
            xpool = ctx.enter_context(tc.tile_pool(name="xpool", bufs=4))
            qt_pool = ctx.enter_context(tc.tile_pool(name="qt", bufs=2))
            kt_pool = ctx.enter_context(tc.tile_pool(name="kt", bufs=2))
            vt_pool = ctx.enter_context(tc.tile_pool(name="vt", bufs=2))
            vaug_pool = ctx.enter_context(tc.tile_pool(name="vaug", bufs=2))
            es_pool = ctx.enter_context(tc.tile_pool(name="es", bufs=4))
            ot_pool = ctx.enter_context(tc.tile_pool(name="ot", bufs=2))
            yep_pool = ctx.enter_context(tc.tile_pool(name="yep", bufs=4))
            rcb_pool = ctx.enter_context(tc.tile_pool(name="rcb", bufs=2))
            bc_pool = ctx.enter_context(tc.tile_pool(name="bc", bufs=4))
            recv_pool = ctx.enter_context(tc.tile_pool(name="recv", bufs=2))
            y_pool = ctx.enter_context(tc.tile_pool(name="y", bufs=3))
            sc_pool = ctx.enter_context(
                tc.tile_pool(name="scp", bufs=2, space="PSUM")
            )
            po_pool = ctx.enter_context(
                tc.tile_pool(name="pop", bufs=2, space="PSUM")
            )
            misc_pool = ctx.enter_context(
                tc.tile_pool(name="mip", bufs=2, space="PSUM")
            )
            dram_in = ctx.enter_context(
                tc.tile_pool(name="a2a_in", bufs=3, space="DRAM")
            )
            dram_out = ctx.enter_context(
                tc.tile_pool(name="a2a_out", bufs=6, space="DRAM")
            )
            dram_bc = ctx.enter_context(
                tc.tile_pool(name="bc_dram", bufs=3, space="DRAM")
            )

            # ---- constants / weights ----
            w_sb = singles.tile([128, CCH, 3 * WC], BF16)
            nc.sync.dma_start(out=w_sb, in_=wqkv)
            identity = singles.tile([128, 128], BF16)
            make_identity(nc, identity)

            proj_consts = {}

            def ensure_proj_consts():
                if proj_consts:
                    return
                wp_sb = singles.tile([128, CCH, c_sz], BF16)
                nc.sync.dma_start(out=wp_sb, in_=wproj)
                b_sb = singles.tile([128, c_sz], F32)
                nc.gpsimd.dma_start(
                    out=b_sb,
                    in_=bass.AP(
                        tensor=bproj.tensor,
                        offset=bproj.offset,
                        ap=[[0, 128]] + list(bproj.ap),
                    ),
                )
                proj_consts["wp"] = wp_sb
                proj_consts["b"] = b_sb

            def chunk_order(b):
                # last batch: process chunks 2..N,0,1 so the tail exposes
                # only chunk 1's AllToAll
                if b == b_sz - 1 and NQC >= 4:
                    return list(range(2, NQC)) + [0, 1]
                return list(range(NQC))

            # per-batch state
            strips = {}       # b -> (QT, KT, VT)
            x_tiles = {}      # (b, rc) -> x_sb
            vaugs = {}        # b -> Vaug
            a2a_chunks = {}   # (b, qc) -> a_out
            oths = {}         # b -> [oTh per head]

            last_b = b_sz - 1

            # ---------- emission helpers ----------

            def emit_strip_group(b, rc, si):
                """One qkv strip-group: (maybe x DMA +) 8 matmuls + copy."""
                if b not in strips:
                    QT = qt_pool.tile([WC, n_sz], BF16, name="QT")
                    KTs = kt_pool.tile([WC, n_sz], BF16, name="KT")
                    VT = vt_pool.tile([WC, n_sz], BF16, name="VT")
                    strips[b] = (QT, KTs, VT)
                QT, KTs, VT = strips[b]
                if (b, rc) not in x_tiles:
                    x_sb = xpool.tile([128, CCH, RCW], BF16)
                    nc.sync.dma_start(out=x_sb, in_=xT[b * NRC + rc])
                    x_tiles[(b, rc)] = x_sb
                x_sb = x_tiles[(b, rc)]
                strip, base = ((VT, 2 * WC), (QT, 0), (KTs, WC))[si]
                ps = misc_pool.tile([128, RCW], F32, tag="mi")
                for cc in range(CCH):
                    nc.tensor.matmul(
                        ps,
                        lhsT=w_sb[:, cc, base:base + WC],
                        rhs=x_sb[:, cc, :],
                        start=(cc == 0),
                        stop=(cc == CCH - 1),
                    )
                nc.vector.tensor_copy(strip[:, rc * RCW:(rc + 1) * RCW], ps)

            def emit_transpose(b, kt):
                """V^T -> V natural for one 128-col k-tile (both heads)."""
                if b not in vaugs:
                    Vaug = vaug_pool.tile([128, HPC * NKT, HD + 1], BF16)
                    nc.vector.memset(Vaug[:, :, HD], 1.0)
                    vaugs[b] = Vaug
                Vaug = vaugs[b]
                VT = strips[b][2]
                pst = misc_pool.tile([128, 128], BF16, tag="mi")
                nc.tensor.transpose(
                    pst, VT[:, kt * 128:(kt + 1) * 128], identity
                )
                nc.vector.tensor_copy(
                    Vaug[:, kt::NKT, 0:HD], pst.rearrange(
                        "p (h d) -> p h d", h=HPC
                    )
                )

            def emit_proj_pair(b, pair, stage):
                """Chunk-pair proj, M=128. stage 0: recv DMA; 1,2: ncol
                matmul group + bias + out DMA."""
                ensure_proj_consts()
                if stage == 0:
                    recv = recv_pool.tile([WC, ncores, 2 * CRB], BF16,
                                          tag="recv")
                    for i in range(2):
                        src = a2a_chunks[(b, 2 * pair + i)]
                        nc.sync.dma_start(
                            out=recv[:, :, i * CRB:(i + 1) * CRB],
                            in_=src.rearrange("j p q -> p j q"),
                        )
                    return recv
                recv, ncol = stage
                wp_sb = proj_consts["wp"]
                b_sb = proj_consts["b"]
                yp = misc_pool.tile([128, 512], F32, tag="mi")
                for j in range(ncores):
                    nc.tensor.matmul(
                        yp,
                        lhsT=recv[:, j, :],
                        rhs=wp_sb[:, j, ncol * 512:(ncol + 1) * 512],
                        start=(j == 0),
                        stop=(j == ncores - 1),
                    )
                y_sb = y_pool.tile([128, 512], F32)
                nc.vector.tensor_add(
                    y_sb, yp, b_sb[:, ncol * 512:(ncol + 1) * 512]
                )
                nc.sync.dma_start(
                    out=out[
                        b,
                        pair * 2 * CRB: (pair + 1) * 2 * CRB,
                        ncol * 512:(ncol + 1) * 512,
                    ],
                    in_=y_sb,
                )

            def emit_proj_single(b, qc, stage):
                """Single-chunk proj, M=64 (used near the tail)."""
                ensure_proj_consts()
                if stage == 0:
                    recv = recv_pool.tile([WC, ncores, CRB], BF16, tag="recv1")
                    src = a2a_chunks[(b, qc)]
                    nc.sync.dma_start(
                        out=recv, in_=src.rearrange("j p q -> p j q")
                    )
                    return recv
                recv, ncol = stage
                wp_sb = proj_consts["wp"]
                b_sb = proj_consts["b"]
                yp = misc_pool.tile([CRB, 512], F32, tag="mi")
                for j in range(ncores):
                    nc.tensor.matmul(
                        yp,
                        lhsT=recv[:, j, :],
                        rhs=wp_sb[:, j, ncol * 512:(ncol + 1) * 512],
                        start=(j == 0),
                        stop=(j == ncores - 1),
                    )
                y_sb = y_pool.tile([CRB, 512], F32)
                nc.vector.tensor_add(
                    y_sb, yp, b_sb[0:CRB, ncol * 512:(ncol + 1) * 512]
                )
                nc.sync.dma_start(
                    out=out[
                        b,
                        qc * CRB:(qc + 1) * CRB,
                        ncol * 512:(ncol + 1) * 512,
                    ],
                    in_=y_sb,
                )

            def emit_boundary(b, qc, po):
                """Chunk epilogue: free PSUM, 1/sums, normalize, ship A2A."""
                # copy po out of PSUM so the next chunk's PV can start
                yc = [yep_pool.tile([HD + 1, QCW], F32, tag="yep",
                                    name=f"yc{h}") for h in range(HPC)]
                for h in range(HPC):
                    nc.vector.tensor_copy(yc[h], po[h])
                # reciprocal of the sums rows (DVE, in place) + pack to a
                # partition-64-aligned tile for the DRAM bounce
                # 1/sums = exp(-ln(sums)) on ACT (both functions pinned to
                # one table set); Exp emits bf16 for the broadcast bounce
                rcb = rcb_pool.tile([HD + 1, HPC, QCW], F32, tag="rcb")
                recb = rcb_pool.tile([HD + 1, HPC, QCW], BF16, tag="recb")
                for h in range(HPC):
                    nc.scalar.activation(
                        rcb[HD:HD + 1, h, :], yc[h][HD:HD + 1, :],
                        mybir.ActivationFunctionType.Ln,
                    )
                    nc.scalar.activation(
                        recb[HD:HD + 1, h, :], rcb[HD:HD + 1, h, :],
                        mybir.ActivationFunctionType.Exp,
                        scale=-1.0,
                    )
                rvd = dram_bc.tile([1, HPC, QCW], BF16, tag="rvd")
                nc.gpsimd.dma_start(out=rvd, in_=recb[HD:HD + 1])
                oth = oths[b]
                for h in range(HPC):
                    bc = bc_pool.tile([HD, QCW], BF16)
                    rvh = rvd[0, h]
                    nc.gpsimd.dma_start(
                        out=bc,
                        in_=bass.AP(
                            tensor=rvh.tensor,
                            offset=rvh.offset,
                            ap=[[0, HD]] + list(rvh.ap),
                        ),
                    )
                    nc.vector.tensor_mul(
                        oth[h][:, qc * QCW:(qc + 1) * QCW], yc[h][0:HD], bc
                    )
                # ship
                a_in = dram_in.tile([ncores, WC, CRB], BF16, tag="ain",
                                    name=f"ain{qc}")
                for h in range(HPC):
                    src = oth[h][:, qc * QCW:(qc + 1) * QCW]
                    nc.sync.dma_start(
                        out=a_in[:, h * HD:(h + 1) * HD, :].rearrange(
                            "j d q -> d j q"
                        ),
                        in_=src.rearrange("d (j q) -> d j q", j=ncores),
                    )
                a_out = dram_out.tile([ncores, WC, CRB], BF16, tag="aout",
                                      name=f"aout{qc}")
                a2a_chunks[(b, qc)] = a_out
                nc.gpsimd.collective_compute(
                    "AllToAll",
                    mybir.AluOpType.bypass,
                    replica_groups=[list(range(ncores))],
                    ins=[a_in.opt()],
                    outs=[a_out.opt()],
                )

            # ---------- filler schedules ----------

            def filler_schedule(b):
                """iteration k -> list of closures."""
                sched = {k: [] for k in range(NIT + 1)}
                if b + 1 < b_sz:
                    ngroups = 3 * NRC
                    lo = 12 if b == 0 else 2
                    hi = max(NIT - 12, lo + ngroups)
                    for g in range(ngroups):
                        rc, si = divmod(g, 3)
                        k = lo + (g * (hi - lo)) // ngroups
                        sched[min(k, NIT)].append(
                            lambda rc=rc, si=si: emit_strip_group(b + 1, rc, si)
                        )
                    q4 = max(NIT // 4, 1)
                    base = max(hi, NIT - q4)  # never before the V strips
                    for kt in range(NKT):
                        k = base + (kt * max(NIT - base, 1)) // NKT
                        sched[min(k, NIT)].append(
                            lambda kt=kt: emit_transpose(b + 1, kt)
                        )

                def add_pair(bp, pair, k0):
                    state = {}

                    def recv(state=state, bp=bp, pair=pair):
                        state["r"] = emit_proj_pair(bp, pair, 0)

                    sched[min(k0, NIT)].append(recv)
                    for ncol in range(NCOL):
                        sched[min(k0 + 2 + 2 * ncol, NIT)].append(
                            lambda ncol=ncol, state=state, bp=bp, pair=pair:
                            emit_proj_pair(bp, pair, (state["r"], ncol))
                        )

                def add_single(bp, qc, k0):
                    state = {}

                    def recv(state=state, bp=bp, qc=qc):
                        state["r"] = emit_proj_single(bp, qc, 0)

                    sched[min(k0, NIT)].append(recv)
                    for ncol in range(NCOL):
                        sched[min(k0 + 2 + 2 * ncol, NIT)].append(
                            lambda ncol=ncol, state=state, bp=bp, qc=qc:
                            emit_proj_single(bp, qc, (state["r"], ncol))
                        )

                npair = NQC // 2
                if b - 1 >= 0:
                    # all of the previous batch's proj pairs, late enough
                    # that the real A2A latency can't head-block the PE
                    # queue (the compile-time scheduler trusts its own
                    # optimistic collective model)
                    for p in range(npair):
                        add_pair(b - 1, npair - 1 - p, 36 + 8 * p)
                if b == last_b:
                    # own chunks: pairs processed (and shipped) early in
                    # this batch, then a single for the second-to-last
                    # processed chunk; the last one goes in the tail
                    order = chunk_order(b)
                    for p in range(npair - 1):
                        qa, qb_ = order[2 * p], order[2 * p + 1]
                        assert qb_ == qa + 1 and qa % 2 == 0
                        # runs inside the tail's A2A wait (free PE time)
                        add_pair(b, qa // 2, NIT - 2 * (npair - 1 - p))
                    if NQC >= 2:
                        add_single(b, order[NQC - 2], NIT - 2)
                return sched

            # ---------- main stream ----------

            # prefetch batch 0's x chunks on parallel DMA queues
            dmaq = [nc.scalar, nc.gpsimd, nc.sync, nc.scalar]
            for rc in range(NRC):
                x_sb = xpool.tile([128, CCH, RCW], BF16)
                dmaq[rc % len(dmaq)].dma_start(out=x_sb, in_=xT[rc])
                x_tiles[(0, rc)] = x_sb
            KPG = NKT // NRC
            for si in range(3):
                emit_strip_group(0, 0, si)
            for kt in range(KPG):
                emit_transpose(0, kt)
            ensure_proj_consts()

            for b in range(b_sz):
                QT, KTs, VT = strips[b]
                Vaug = vaugs[b]
                oths[b] = [ot_pool.tile([HD, n_sz], BF16, name=f"oTh{h}")
                           for h in range(HPC)]
                sched = filler_schedule(b)
                if b == 0:
                    # rest of batch 0's own qkv, paced just ahead of use
                    for r in range(1, NRC):
                        for si in range(3):
                            sched[3 * (r - 1) + si].insert(
                                si, (lambda r=r, si=si:
                                     emit_strip_group(0, r, si)))
                        for kt in range(KPG * r, KPG * (r + 1)):
                            sched[3 * r].insert(
                                3, lambda kt=kt: emit_transpose(0, kt))
                es_hist = {}
                po_cur = None

                qc_ord = chunk_order(b)

                def emit_pv(k):
                    nonlocal po_cur
                    qp, ktp = divmod(k, NKT)
                    qp = qc_ord[qp]
                    if ktp == 0:
                        po_cur = po_new[0]
                    es_prev = es_hist.pop(k)
                    for h in range(HPC):
                        nc.tensor.matmul(
                            po_cur[h],
                            lhsT=Vaug[:, h * NKT + ktp, :],
                            rhs=es_prev[:, h, :],
                            start=(ktp == 0), stop=(ktp == NKT - 1),
                        )
                    if ktp == NKT - 1:
                        emit_boundary(b, qp, po_cur)

                def emit_scores(k):
                    qc, kt = divmod(k, NKT)
                    qc = qc_ord[qc]
                    if kt == 0:
                        po_new[0] = [po_pool.tile([HD + 1, QCW], F32,
                                                  tag="po", name=f"po{h}")
                                     for h in range(HPC)]
                    sc = sc_pool.tile([128, HPC, QCW], F32, tag="sc")
                    for h in range(HPC):
                        nc.tensor.matmul(
                            sc[:, h, :],
                            lhsT=KTs[h * HD:(h + 1) * HD,
                                     kt * 128:(kt + 1) * 128],
                            rhs=QT[h * HD:(h + 1) * HD,
                                   qc * QCW:(qc + 1) * QCW],
                            start=True, stop=True,
                        )
                    sc_hist[k] = sc

                def emit_exp(k):
                    es = es_pool.tile([128, HPC, QCW], BF16, tag="es")
                    nc.scalar.activation(
                        es, sc_hist.pop(k), mybir.ActivationFunctionType.Exp,
                        scale=SCALE,
                    )
                    es_hist[k] = es

                # iterations processed in groups of 2 so the PE runs 4
                # same-geometry score matmuls, then 4 PVs (geometry
                # switches flush the weight-load pipeline)
                po_new = [None]
                sc_hist = {}
                for g in range(NIT // 2 + 1):
                    kE = 2 * g
                    boundary = kE >= 1 and kE % NKT == 0
                    if boundary:
                        # the chunk's last PVs + epilogue/ship go FIRST so
                        # the A2A chain starts before this group's scores
                        emit_pv(kE - 2)
                        emit_pv(kE - 1)
                    for k in (kE, kE + 1):
                        if k < NIT:
                            emit_scores(k)
                    for k in (kE, kE + 1):
                        if k < NIT:
                            emit_exp(k)
                    if not boundary:
                        for kp in (kE - 2, kE - 1):
                            if kp >= 0:
                                emit_pv(kp)
                    for k in (kE, kE + 1):
                        if k <= NIT:
                            for fn in sched[k]:
                                fn()

                for rc in range(NRC):
                    x_tiles.pop((b, rc), None)

            # ---------- tail: final chunk's proj ----------
            last_qc = chunk_order(last_b)[NQC - 1]
            recv = emit_proj_single(last_b, last_qc, 0)
            for ncol in range(NCOL):
                emit_proj_single(last_b, last_qc, (recv, ncol))

    nc.compile()
    return nc


def shard_inputs(x, w_qkv, w_proj, b_proj, b_sz=B, n_sz=N, c_sz=C, h_sz=H,
                 ncores=NCORES):
    """Build per-core input maps from the full inputs."""
    import ml_dtypes

    mm_np = ml_dtypes.bfloat16
    HPC = h_sz // ncores
    HD = c_sz // h_sz
    x = np.asarray(x, dtype=np.float32)
    w_qkv = np.asarray(w_qkv, dtype=np.float32).astype(mm_np)
    w_proj = np.ascontiguousarray(np.asarray(w_proj, dtype=np.float32)
                                  .astype(mm_np))
    b_proj = np.ascontiguousarray(np.asarray(b_proj, dtype=np.float32))

    RCW = 512
    NRC = n_sz // RCW
    CCH = c_sz // 128
    # x chunk (b, rc) pre-transposed to [128, CCH, RCW], chunk-contiguous
    xT = x.reshape(b_sz * n_sz, c_sz).T.astype(mm_np)          # [C, B*N]
    xTc = np.ascontiguousarray(
        xT.reshape(CCH, 128, b_sz * NRC, RCW).transpose(2, 1, 0, 3)
    )
    wp_c = np.ascontiguousarray(
        w_proj.reshape(CCH, 128, c_sz).transpose(1, 0, 2)
    )
    w4 = w_qkv.reshape(c_sz, 3, h_sz, HD)
    in_maps = []
    for c in range(ncores):
        wc = w4[:, :, c * HPC:(c + 1) * HPC, :].reshape(c_sz, 3 * HPC * HD)
        wc_c = np.ascontiguousarray(
            wc.reshape(CCH, 128, 3 * HPC * HD).transpose(1, 0, 2)
        )
        in_maps.append(
            {"xT": xTc, "wqkv": wc_c, "wproj": wp_c, "bproj": b_proj}
        )
    return in_maps


def assemble_output(results, b_sz=B, n_sz=N, c_sz=C, ncores=NCORES):
    RB = n_sz // ncores
    QCW = 512
    NQC = n_sz // QCW
    CRB = RB // NQC
    full = np.empty((b_sz, n_sz, c_sz), dtype=np.float32)
    for r in range(ncores):
        o = results[r]["out"]
        for b in range(b_sz):
            for qc in range(NQC):
                full[b, qc * QCW + r * CRB: qc * QCW + (r + 1) * CRB, :] \
                    = o[b, qc * CRB:(qc + 1) * CRB, :]
    return full


def run(x, w_qkv, w_proj, b_proj, trace=False, **run_kwargs):
    nc = build_nc()
    in_maps = shard_inputs(x, w_qkv, w_proj, b_proj)
    last_err = None
    for attempt in range(3):
        try:
            res = run_bass_kernel_spmd(
                nc, in_maps, core_ids=list(range(NCORES)), trace=trace,
                **run_kwargs
            )
            return assemble_output(res.results), res
        except Exception as e:  # transient device wedges happen; retry
            last_err = e
            import time
            time.sleep(10)
    raise last_err


def kernel(x, w_qkv, w_proj, b_proj):
    out, _ = run(x, w_qkv, w_proj, b_proj)
    return out
